# revision 22
# baseline (speedup 1.0000x reference)
"""Trainium2 Bass kernel for nn_Mlp_moe: dense patch-token MLP + top-1 gated
atom (expert) routing for 6 CLS task tokens.

Sharding over 8 NeuronCores:
  - Patch MLP: data-parallel over batch B=64 -> 8 batches (1568 patch tokens)
    per core. MLP weights replicated (SBUF-resident, bf16).
  - Atom/CLS part: hidden dim H=3072 sharded 8-way (384 per core); every core
    processes all 384 CLS tokens on its H-shard and emits a partial output
    summed on the host. Routing is computed on the host and shipped as a
    combined src-select*gate-weight mask folded into the device compute.

Atom work restructure: each task slot n has exactly TWO candidate (src,dst)
atom pairs (left: (n//2, 3+n%2), right: (3+n%2, n//2)), so with slot-major
token ordering each atom's candidate token set is a STATIC block pattern:
  in-atom 0,1,2 <- slots {2a,2a+1} (128 contiguous cols)
  in-atom 3,4   <- even/odd slots  (3x64 cols, stride 128)
The in/out GEMMs run only over candidate columns (768 instead of 5*384=1920),
with strided access patterns; the host mask m_a[t] = w(t)*[src(t)==a] both
selects the chosen expert and applies the gate weight.

Startup: patch GEMM1 deps (x chunk 0 + w1 in eighths) load first with minimal
queue competition so the PE starts ~10us in; the remaining tensors stream
behind on the sync ring in need order (w2 sixths for the h-outer GEMM2, then
cls/ain/aout for the atom phases, then x2/x3). A few dummy matmuls on a
zeroed tile warm the PE HAM clock gate during the initial DMA wait. The atom
phases slot between patch chunks so every dependency is resident well before
its consumer. The last chunk's GEMM2 runs dp-outer so its outputs drain
progressively instead of all at chunk end.

Device compute is bf16 (PSUM accumulation fp32, erf-Gelu on ScalarE);
outputs fp32. Tokens ride the free dim; 4 chunks of 392 columns.
"""

import numpy as np
import ml_dtypes

import concourse.bass as bass
import concourse.bacc as bacc
import concourse.mybir as mybir
from concourse import tile
from concourse.bass_utils import run_bass_kernel_spmd

NCORES = 8
B, NCLS, P, D, H = 64, 6, 196, 768, 3072
NA = 5
HSH = H // NCORES            # 384: per-core atom hidden shard
BPC = B // NCORES            # 8 batches per core
TPC = BPC * P                # 1568 patch tokens per core
NT = B * NCLS                # 384 cls tokens
DT = D // 128                # 6 d-tiles
HT = H // 128                # 24 h-tiles
HLT = NA * HSH // 128        # 15 atom h-shard tiles (a-major, 3 per atom)
KPA = HSH // 128             # 3 h-shard tiles per atom
CW = 392
NCH = 4
CHUNKS = [(i * CW, CW) for i in range(NCH)]
W1E = HT * DT * 128 // 12    # w1 twelfth cols (2 h-tiles)
W2G = HT * D // 6            # w2 sixth cols (4 h-tiles)

# candidate-atom layout in the 768-wide hidden tiles (per k):
#   atom a offset OFF[a], width SA[a]
OFF = [0, 128, 256, 384, 576]
SA = [128, 128, 128, 192, 192]

LEFT_KEYS = np.array([3, 4, 8, 9, 13, 14], dtype=np.int64)
RIGHT_KEYS = np.array([15, 20, 16, 21, 17, 22], dtype=np.int64)

BF16 = mybir.dt.bfloat16
F32 = mybir.dt.float32
AF = mybir.ActivationFunctionType

_CACHE = {}
LAST_RESULTS = None  # BassKernelResults of the most recent run (for profiling)


def _build_program():
    nc = bacc.Bacc(None, target_bir_lowering=False, debug=False,
                   num_devices=NCORES)

    # partition-major packed inputs (see host layouts in kernel())
    xT_d = nc.dram_tensor("xT", [128, NCH * DT * CW], BF16,
                          kind="ExternalInput")
    w1T_d = nc.dram_tensor("w1T", [128, HT * DT * 128], BF16,
                           kind="ExternalInput")
    b1T_d = nc.dram_tensor("b1T", [128, HT], F32, kind="ExternalInput")
    w2T_d = nc.dram_tensor("w2T", [128, HT * D], BF16, kind="ExternalInput")
    clsT_d = nc.dram_tensor("clsT", [128, DT * NT], BF16,
                            kind="ExternalInput")
    ainT_d = nc.dram_tensor("ainT", [DT, 128, NA * HSH], BF16,
                            kind="ExternalInput")
    ainbT_d = nc.dram_tensor("ainbT", [128, HLT], F32, kind="ExternalInput")
    aoutT_d = nc.dram_tensor("aoutT", [NA, 128, KPA * D], BF16,
                             kind="ExternalInput")
    mW_d = nc.dram_tensor("mW", [128, 2 * NT], F32, kind="ExternalInput")
    poutT_d = nc.dram_tensor("poutT", [DT, 128, TPC], F32,
                             kind="ExternalOutput")
    cpartT_d = nc.dram_tensor("cpartT", [DT, 128, NT], F32,
                              kind="ExternalOutput")

    with tile.TileContext(nc) as tc:
        with (
            tc.tile_pool(name="w", bufs=1) as wp,
            tc.tile_pool(name="gat", bufs=1) as gp,
            tc.tile_pool(name="xin", bufs=3) as xp,
            tc.tile_pool(name="g1", bufs=24) as g1p,
            tc.tile_pool(name="ostg", bufs=8) as op,
            tc.tile_pool(name="ps", bufs=8, space="PSUM") as pp,
        ):
            # ---- resident tiles ----
            w1T = wp.tile([128, HT * DT * 128], BF16, tag="w1", name="w1")
            w2T = wp.tile([128, HT * D], BF16, tag="w2", name="w2")
            b1T = wp.tile([128, HT], F32, tag="b1", name="b1")
            clsT = wp.tile([128, DT * NT], BF16, tag="cls", name="cls")
            ainbT = wp.tile([128, HLT], F32, tag="ainb", name="ainb")
            mW = wp.tile([128, 2 * NT], F32, tag="mW", name="mW")
            ainT = [wp.tile([128, NA * HSH], BF16, tag=f"ain{d}",
                            name=f"ain{d}") for d in range(DT)]
            aoutT = [wp.tile([128, KPA * D], BF16, tag=f"ao{a}",
                             name=f"ao{a}") for a in range(NA)]
            warm = wp.tile([128, 640], BF16, tag="warm", name="warm")

            # ---- DMA issue, need-ordered ----
            def load_x(ci, eng):
                xa = xp.tile([128, DT * CW], BF16, tag="x", name="x")
                eng.dma_start(
                    xa[:], xT_d[:, ci * DT * CW:(ci + 1) * DT * CW])
                return xa

            # scalar queue: chunk-0 GEMM1/atom small deps only, then gelus;
            # x0 is split across scalar+gpsimd so both halves land in ~2.5us
            x0 = xp.tile([128, DT * CW], BF16, tag="x", name="x")
            XH = DT * CW // 2
            nc.scalar.dma_start(x0[:, :XH], xT_d[:, :XH])
            nc.gpsimd.dma_start(x0[:, XH:], xT_d[:, XH:DT * CW])
            nc.scalar.dma_start(b1T[:], b1T_d[:])
            nc.scalar.dma_start(ainbT[:], ainbT_d[:])
            nc.scalar.dma_start(mW[:], mW_d[:])

            # sync ring (FIFO): w1 h0 single -> w1 twelfths -> w2 sixths ->
            # cls/ain/aout -> x2/x3.  Drains in exactly this need order.
            W1H = DT * 128
            nc.sync.dma_start(w1T[:, :W1H], w1T_d[:, :W1H])
            nc.sync.dma_start(w1T[:, W1H:W1E], w1T_d[:, W1H:W1E])
            for e in range(1, 12):
                nc.sync.dma_start(w1T[:, e * W1E:(e + 1) * W1E],
                                  w1T_d[:, e * W1E:(e + 1) * W1E])
            for g in range(6):
                nc.sync.dma_start(w2T[:, g * W2G:(g + 1) * W2G],
                                  w2T_d[:, g * W2G:(g + 1) * W2G])
            nc.sync.dma_start(clsT[:], clsT_d[:])
            for d in range(DT):
                nc.sync.dma_start(ainT[d][:], ainT_d[d])
            for a in range(NA):
                nc.sync.dma_start(aoutT[a][:], aoutT_d[a])
            x2 = load_x(2, nc.sync)
            x3 = load_x(3, nc.sync)

            # gpsimd queue: x0 tail-half, x1, then carries most output DMAs
            x1 = load_x(1, nc.gpsimd)
            xs = [x0, x1, x2, x3]

            # ---- PE HAM warm-up during the initial DMA wait ----
            nc.vector.memset(warm[:], 0)
            wps = pp.tile([128, 512], F32, tag="ps", name="ps")
            for i in range(9):
                nc.tensor.matmul(wps[:, :512], warm[:, :128], warm[:, 128:],
                                 start=True, stop=True)

            # strided candidate views of cls: [128, d, b(l/r-parity), s, 64]
            cls_r = clsT[:].rearrange("p (d s b x) -> p d b s x",
                                      d=DT, s=3, b=2, x=64)

            def patch_g1(ci, xa, h0, h1, g1s):
                c0, cw = CHUNKS[ci]
                for h in range(h0, h1):
                    ps = pp.tile([128, 512], F32, tag="ps", name="ps")
                    for d in range(DT):
                        nc.tensor.matmul(
                            ps[:, :cw],
                            w1T[:, (h * DT + d) * 128:(h * DT + d + 1) * 128],
                            xa[:, d * CW:d * CW + cw],
                            start=(d == 0), stop=(d == DT - 1))
                    g1 = g1p.tile([128, CW], BF16, tag="g1", name="g1")
                    nc.scalar.activation(g1[:, :cw], ps[:, :cw], AF.Gelu,
                                         bias=b1T[:, h:h + 1])
                    g1s.append(g1)
                return g1s

            def patch_g2(ci, g1s, eng_rr, dp_outer=False):
                c0, cw = CHUNKS[ci]

                def flush(dp, ps):
                    stg = op.tile([128, CW], F32, tag="ostg", name="ostg")
                    nc.vector.tensor_copy(stg[:, :cw], ps[:, :cw])
                    eng_rr[dp % len(eng_rr)].dma_start(
                        poutT_d[dp][:, c0:c0 + cw], stg[:, :cw])

                if dp_outer:
                    for dp in range(DT):
                        ps = pp.tile([128, 512], F32, tag="ps", name="ps")
                        for h in range(HT):
                            nc.tensor.matmul(
                                ps[:, :cw],
                                w2T[:, h * D + dp * 128:
                                    h * D + (dp + 1) * 128],
                                g1s[h][:, :cw],
                                start=(h == 0), stop=(h == HT - 1))
                        flush(dp, ps)
                else:
                    ps2 = [pp.tile([128, 512], F32, tag="ps", name="ps")
                           for _ in range(DT)]
                    for h in range(HT):
                        for dp in range(DT):
                            nc.tensor.matmul(
                                ps2[dp][:, :cw],
                                w2T[:, h * D + dp * 128:
                                    h * D + (dp + 1) * 128],
                                g1s[h][:, :cw],
                                start=(h == 0), stop=(h == HT - 1))
                    for dp in range(DT):
                        flush(dp, ps2[dp])

            # ---- chunk 0 (GEMM2 h-outer: w2 streams behind it) ----
            g1s0 = patch_g1(0, x0, 0, HT, [])
            patch_g2(0, g1s0, [nc.gpsimd])

            # ---- atom in-GEMM over candidate columns + gelu + mask ----
            gk = [gp.tile([128, 2 * NT], BF16, tag=f"g{k}", name=f"g{k}")
                  for k in range(KPA)]
            hk = [gp.tile([128, 2 * NT], BF16, tag=f"h{k}", name=f"h{k}")
                  for k in range(KPA)]
            for a in range(NA):
                for k in range(KPA):
                    ps = pp.tile([128, 512], F32, tag="ps", name="ps")
                    for d in range(DT):
                        if a < 3:
                            rhs = clsT[:, d * NT + 128 * a:
                                       d * NT + 128 * a + 128]
                        else:
                            rhs = cls_r[:, d, a - 3]
                        nc.tensor.matmul(
                            ps[:, :SA[a]],
                            ainT[d][:, a * HSH + k * 128:
                                    a * HSH + (k + 1) * 128],
                            rhs, start=(d == 0), stop=(d == DT - 1))
                    hl = a * KPA + k
                    nc.scalar.activation(gk[k][:, OFF[a]:OFF[a] + SA[a]],
                                         ps[:, :SA[a]], AF.Gelu,
                                         bias=ainbT[:, hl:hl + 1])
            for k in range(KPA):
                nc.vector.tensor_mul(hk[k][:], gk[k][:], mW[:])

            # ---- chunk 1 GEMM1 head (covers the atom gelu/mask latency) ----
            g1s1 = patch_g1(1, x1, 0, 6, [])

            # ---- atom out-GEMM ----
            # bank A: out-atoms 3,4 consume h~_{0,1,2} parity halves
            #   (compact cols: [b=0: s*64 | b=1: 192 + s*64])
            # bank B: out-atoms 0,1,2 consume h~_{3,4} slot-index columns
            #   (token-order cols 128*a)
            hA = [hk[k][:, :NT].rearrange("p (s b x) -> p b s x",
                                          s=3, b=2, x=64) for k in range(KPA)]
            hB = [hk[k][:, NT:].rearrange("p (u s x) -> p s u x",
                                          u=2, s=3, x=64) for k in range(KPA)]
            for dp in range(DT):
                psA = pp.tile([128, 512], F32, tag="ps", name="ps")
                psB = pp.tile([128, 512], F32, tag="ps", name="ps")
                for a, half in ((3, 0), (4, 1)):
                    for k in range(KPA):
                        nc.tensor.matmul(
                            psA[:, half * 192:(half + 1) * 192],
                            aoutT[a][:, k * D + dp * 128:
                                     k * D + (dp + 1) * 128],
                            hA[k][:, half],
                            start=(k == 0), stop=(k == KPA - 1))
                for a in range(3):
                    for k in range(KPA):
                        nc.tensor.matmul(
                            psB[:, a * 128:(a + 1) * 128],
                            aoutT[a][:, k * D + dp * 128:
                                     k * D + (dp + 1) * 128],
                            hB[k][:, a],
                            start=(k == 0), stop=(k == KPA - 1))
                stg = op.tile([128, CW], F32, tag="ostg", name="ostg")
                # token order (2s+b)*64+x <- psA[b*192+s*64+x] + psB[token]
                # (two DVE ops: the verifier rejects dual-PSUM-input ops)
                outv = stg[:, :NT].rearrange("p (s b x) -> p s b x",
                                             s=3, b=2, x=64)
                av = psA[:, :NT].rearrange("p (b s x) -> p s b x",
                                           b=2, s=3, x=64)
                nc.vector.tensor_copy(outv, av)
                nc.vector.tensor_add(stg[:, :NT], stg[:, :NT], psB[:, :NT])
                nc.gpsimd.dma_start(cpartT_d[dp], stg[:, :NT])

            # ---- remaining patch chunks ----
            patch_g1(1, x1, 6, HT, g1s1)
            patch_g2(1, g1s1, [nc.gpsimd])
            g1s2 = patch_g1(2, x2, 0, HT, [])
            patch_g2(2, g1s2, [nc.gpsimd])
            g1s3 = patch_g1(3, x3, 0, HT, [])
            patch_g2(3, g1s3, [nc.gpsimd, nc.sync, nc.scalar],
                     dp_outer=True)

    nc.compile()
    return nc


def _sigmoid(x):
    out = np.empty_like(x)
    pos = x >= 0
    out[pos] = 1.0 / (1.0 + np.exp(-x[pos]))
    ex = np.exp(x[~pos])
    out[~pos] = ex / (1.0 + ex)
    return out


def kernel(x, patch_w1, patch_b1, patch_w2, patch_b2, gate_delta,
           atom_in_w, atom_in_b, atom_out_w, atom_out_b):
    x = np.asarray(x, dtype=np.float32)
    patch_w1 = np.asarray(patch_w1, dtype=np.float32)
    patch_b1 = np.asarray(patch_b1, dtype=np.float32)
    patch_w2 = np.asarray(patch_w2, dtype=np.float32)
    patch_b2 = np.asarray(patch_b2, dtype=np.float32)
    gate_delta = np.asarray(gate_delta, dtype=np.float32)
    atom_in_w = np.asarray(atom_in_w, dtype=np.float32)
    atom_in_b = np.asarray(atom_in_b, dtype=np.float32)
    atom_out_w = np.asarray(atom_out_w, dtype=np.float32)
    atom_out_b = np.asarray(atom_out_b, dtype=np.float32)

    bf = ml_dtypes.bfloat16

    # ---- host routing (tiny), slot-major token order t' = n*64 + b ----
    cls3 = x[:, :NCLS, :]                                   # [B, 6, D]
    logits = np.einsum("bnd,nd->bn", cls3, gate_delta)      # [B, 6] f32
    choose_left = logits >= 0
    p_left = _sigmoid(logits)
    wgt = np.where(choose_left, p_left, 1.0 - p_left).astype(np.float32)
    keys = np.where(choose_left, LEFT_KEYS[None, :], RIGHT_KEYS[None, :])
    src_sm = (keys // NA).T.reshape(-1)                     # [384] slot-major
    dst_sm = (keys % NA).T.reshape(-1)
    w_sm = wgt.T.reshape(-1).astype(np.float32)

    # combined src-select * gate-weight mask over candidate columns
    SLOTS = {0: [0, 1], 1: [2, 3], 2: [4, 5], 3: [0, 2, 4], 4: [1, 3, 5]}
    m = np.zeros((2 * NT,), dtype=np.float32)
    for a in range(NA):
        cols = np.concatenate(
            [np.arange(s * 64, (s + 1) * 64) for s in SLOTS[a]])
        m[OFF[a]:OFF[a] + SA[a]] = np.where(
            src_sm[cols] == a, w_sm[cols], 0.0)
    mW_rep = np.ascontiguousarray(
        np.broadcast_to(m[None, :], (128, 2 * NT))).astype(np.float32)

    # ---- replicated tensors (partition-major packed) ----
    cls_sm = np.ascontiguousarray(cls3.transpose(1, 0, 2)).reshape(NT, D)
    # clsT[p, d*NT + t'] = cls_sm[t', d*128+p]
    clsT = np.ascontiguousarray(
        cls_sm.reshape(NT, DT, 128).transpose(2, 1, 0)
    ).reshape(128, DT * NT).astype(bf)
    # w1T[p, (h*6+d)*128 + m] = patch_w1[h*128+m, d*128+p]
    w1T = np.ascontiguousarray(
        patch_w1.reshape(HT, 128, DT, 128).transpose(3, 0, 2, 1)
    ).reshape(128, HT * DT * 128).astype(bf)
    b1T = np.ascontiguousarray(patch_b1.reshape(HT, 128).T)
    # w2T[p, h*D + dp*128 + m] = patch_w2[dp*128+m, h*128+p]
    w2T = np.ascontiguousarray(
        patch_w2.reshape(DT, 128, HT, 128).transpose(3, 2, 0, 1)
    ).reshape(128, HT * D).astype(bf)

    # ---- per-core tensors ----
    patch = x[:, NCLS:, :].reshape(NCORES, TPC, D)
    # xT[p, ci*DT*CW + d*CW + t] = patch[c][ci*CW+t, d*128+p]
    xT_all = np.ascontiguousarray(
        patch.reshape(NCORES, NCH, CW, DT, 128).transpose(0, 4, 1, 3, 2)
    ).reshape(NCORES, 128, NCH * DT * CW).astype(bf)

    ainT_all, ainbT_all, aoutT_all = [], [], []
    for c in range(NCORES):
        hsl = slice(HSH * c, HSH * (c + 1))
        # ainT[d, p, a*HSH + k*128 + m] = atom_in_w[a, hsl0 + k*128+m, d*128+p]
        ainT = np.ascontiguousarray(
            atom_in_w[:, hsl, :].reshape(NA, KPA, 128, DT, 128)
            .transpose(3, 4, 0, 1, 2)).reshape(DT, 128, NA * HSH).astype(bf)
        ainT_all.append(ainT)
        ainbT_all.append(np.ascontiguousarray(
            atom_in_b[:, hsl].reshape(HLT, 128).T))
        # aoutT[a, p, k*D + dp*128 + m] = atom_out_w[a, dp*128+m, hsl0+k*128+p]
        aoutT = np.ascontiguousarray(
            atom_out_w[:, :, hsl].reshape(NA, DT, 128, KPA, 128)
            .transpose(0, 4, 3, 1, 2)).reshape(NA, 128, KPA * D).astype(bf)
        aoutT_all.append(aoutT)

    in_maps = []
    for c in range(NCORES):
        in_maps.append({
            "xT": xT_all[c], "w1T": w1T, "b1T": b1T, "w2T": w2T,
            "clsT": clsT, "ainT": ainT_all[c], "ainbT": ainbT_all[c],
            "aoutT": aoutT_all[c], "mW": mW_rep,
        })

    nc = _CACHE.get("nc")
    if nc is None:
        nc = _build_program()
        _CACHE["nc"] = nc

    res = run_bass_kernel_spmd(nc, in_maps, core_ids=list(range(NCORES)))
    global LAST_RESULTS
    LAST_RESULTS = res

    # ---- host gather ----
    patch_out = np.empty((B, P, D), dtype=np.float32)
    for c in range(NCORES):
        poutT = res.results[c]["poutT"].reshape(D, TPC)
        patch_out[BPC * c:BPC * (c + 1)] = (
            poutT.T + patch_b2[None, :]).reshape(BPC, P, D)

    cpart = np.zeros((D, NT), dtype=np.float32)
    for c in range(NCORES):
        cpart += res.results[c]["cpartT"].reshape(D, NT)
    cls_out_sm = cpart.T + w_sm[:, None] * atom_out_b[dst_sm, :]
    cls_out = cls_out_sm.reshape(NCLS, B, D).transpose(1, 0, 2)

    return np.concatenate([cls_out, patch_out], axis=1)


# revision 24
# speedup vs baseline: 1.0012x; 1.0012x over previous
"""Trainium2 Bass kernel for nn_Mlp_moe: dense patch-token MLP + top-1 gated
atom (expert) routing for 6 CLS task tokens.

Sharding over 8 NeuronCores:
  - Patch MLP: data-parallel over batch B=64 -> 8 batches (1568 patch tokens)
    per core. MLP weights replicated (SBUF-resident, bf16).
  - Atom/CLS part: hidden dim H=3072 sharded 8-way (384 per core); every core
    processes all 384 CLS tokens on its H-shard and emits a partial output
    summed on the host. Routing is computed on the host and shipped as a
    combined src-select*gate-weight mask folded into the device compute.

Atom work restructure: each task slot n has exactly TWO candidate (src,dst)
atom pairs (left: (n//2, 3+n%2), right: (3+n%2, n//2)), so with slot-major
token ordering each atom's candidate token set is a STATIC block pattern:
  in-atom 0,1,2 <- slots {2a,2a+1} (128 contiguous cols)
  in-atom 3,4   <- even/odd slots  (3x64 cols, stride 128)
The in/out GEMMs run only over candidate columns (768 instead of 5*384=1920),
with strided access patterns; the host mask m_a[t] = w(t)*[src(t)==a] both
selects the chosen expert and applies the gate weight.

Startup: patch GEMM1 deps (x chunk 0 + w1 in eighths) load first with minimal
queue competition so the PE starts ~10us in; the remaining tensors stream
behind on the sync ring in need order (w2 sixths for the h-outer GEMM2, then
cls/ain/aout for the atom phases, then x2/x3). A few dummy matmuls on a
zeroed tile warm the PE HAM clock gate during the initial DMA wait. The atom
phases slot between patch chunks so every dependency is resident well before
its consumer. The last chunk's GEMM2 runs dp-outer so its outputs drain
progressively instead of all at chunk end.

Device compute is bf16 (PSUM accumulation fp32, erf-Gelu on ScalarE);
outputs fp32. Tokens ride the free dim; 4 chunks of 392 columns.
"""

import numpy as np
import ml_dtypes

import concourse.bass as bass
import concourse.bacc as bacc
import concourse.mybir as mybir
from concourse import tile
from concourse.bass_utils import run_bass_kernel_spmd

NCORES = 8
B, NCLS, P, D, H = 64, 6, 196, 768, 3072
NA = 5
HSH = H // NCORES            # 384: per-core atom hidden shard
BPC = B // NCORES            # 8 batches per core
TPC = BPC * P                # 1568 patch tokens per core
NT = B * NCLS                # 384 cls tokens
DT = D // 128                # 6 d-tiles
HT = H // 128                # 24 h-tiles
HLT = NA * HSH // 128        # 15 atom h-shard tiles (a-major, 3 per atom)
KPA = HSH // 128             # 3 h-shard tiles per atom
CW = 392
NCH = 4
CHUNKS = [(i * CW, CW) for i in range(NCH)]
W1E = HT * DT * 128 // 12    # w1 twelfth cols (2 h-tiles)
W2G = HT * D // 6            # w2 sixth cols (4 h-tiles)

# candidate-atom layout in the 768-wide hidden tiles (per k):
#   atom a offset OFF[a], width SA[a]
OFF = [0, 128, 256, 384, 576]
SA = [128, 128, 128, 192, 192]

LEFT_KEYS = np.array([3, 4, 8, 9, 13, 14], dtype=np.int64)
RIGHT_KEYS = np.array([15, 20, 16, 21, 17, 22], dtype=np.int64)

BF16 = mybir.dt.bfloat16
F32 = mybir.dt.float32
AF = mybir.ActivationFunctionType

_CACHE = {}
LAST_RESULTS = None  # BassKernelResults of the most recent run (for profiling)


def _build_program():
    nc = bacc.Bacc(None, target_bir_lowering=False, debug=False,
                   num_devices=NCORES)

    # partition-major packed inputs (see host layouts in kernel())
    xT_d = nc.dram_tensor("xT", [128, NCH * DT * CW], BF16,
                          kind="ExternalInput")
    w1T_d = nc.dram_tensor("w1T", [128, HT * DT * 128], BF16,
                           kind="ExternalInput")
    b1T_d = nc.dram_tensor("b1T", [128, HT], F32, kind="ExternalInput")
    w2T_d = nc.dram_tensor("w2T", [128, HT * D], BF16, kind="ExternalInput")
    clsT_d = nc.dram_tensor("clsT", [128, DT * NT], BF16,
                            kind="ExternalInput")
    ainT_d = nc.dram_tensor("ainT", [DT, 128, NA * HSH], BF16,
                            kind="ExternalInput")
    ainbT_d = nc.dram_tensor("ainbT", [128, HLT], F32, kind="ExternalInput")
    aoutT_d = nc.dram_tensor("aoutT", [NA, 128, KPA * D], BF16,
                             kind="ExternalInput")
    mW_d = nc.dram_tensor("mW", [128, 2 * NT], F32, kind="ExternalInput")
    poutT_d = nc.dram_tensor("poutT", [DT, 128, TPC], F32,
                             kind="ExternalOutput")
    cpartT_d = nc.dram_tensor("cpartT", [DT, 128, NT], F32,
                              kind="ExternalOutput")

    with tile.TileContext(nc) as tc:
        with (
            tc.tile_pool(name="w", bufs=1) as wp,
            tc.tile_pool(name="gat", bufs=1) as gp,
            tc.tile_pool(name="xin", bufs=3) as xp,
            tc.tile_pool(name="g1", bufs=24) as g1p,
            tc.tile_pool(name="ostg", bufs=8) as op,
            tc.tile_pool(name="ps", bufs=8, space="PSUM") as pp,
        ):
            # ---- resident tiles ----
            w1T = wp.tile([128, HT * DT * 128], BF16, tag="w1", name="w1")
            w2T = wp.tile([128, HT * D], BF16, tag="w2", name="w2")
            b1T = wp.tile([128, HT], F32, tag="b1", name="b1")
            clsT = wp.tile([128, DT * NT], BF16, tag="cls", name="cls")
            ainbT = wp.tile([128, HLT], F32, tag="ainb", name="ainb")
            mW = wp.tile([128, 2 * NT], F32, tag="mW", name="mW")
            ainT = [wp.tile([128, NA * HSH], BF16, tag=f"ain{d}",
                            name=f"ain{d}") for d in range(DT)]
            aoutT = [wp.tile([128, KPA * D], BF16, tag=f"ao{a}",
                             name=f"ao{a}") for a in range(NA)]
            warm = wp.tile([128, 640], BF16, tag="warm", name="warm")

            # ---- DMA issue, need-ordered ----
            def load_x(ci, eng):
                xa = xp.tile([128, DT * CW], BF16, tag="x", name="x")
                eng.dma_start(
                    xa[:], xT_d[:, ci * DT * CW:(ci + 1) * DT * CW])
                return xa

            # scalar queue: chunk-0 GEMM1/atom small deps only, then gelus;
            # x0 is split across scalar+gpsimd so both halves land in ~2.5us
            x0 = xp.tile([128, DT * CW], BF16, tag="x", name="x")
            XH = DT * CW // 2
            nc.scalar.dma_start(x0[:, :XH], xT_d[:, :XH])
            nc.gpsimd.dma_start(x0[:, XH:], xT_d[:, XH:DT * CW])
            nc.scalar.dma_start(b1T[:], b1T_d[:])
            nc.scalar.dma_start(ainbT[:], ainbT_d[:])
            nc.scalar.dma_start(mW[:], mW_d[:])

            # sync ring (FIFO): w1 twelfths -> w2 sixths -> cls/ain/aout ->
            # x2/x3.  Everything drains in exactly this need order.
            for e in range(12):
                nc.sync.dma_start(w1T[:, e * W1E:(e + 1) * W1E],
                                  w1T_d[:, e * W1E:(e + 1) * W1E])
            for g in range(6):
                nc.sync.dma_start(w2T[:, g * W2G:(g + 1) * W2G],
                                  w2T_d[:, g * W2G:(g + 1) * W2G])
            nc.sync.dma_start(clsT[:], clsT_d[:])
            for d in range(DT):
                nc.sync.dma_start(ainT[d][:], ainT_d[d])
            for a in range(NA):
                nc.sync.dma_start(aoutT[a][:], aoutT_d[a])
            x2 = load_x(2, nc.sync)
            x3 = load_x(3, nc.sync)

            # gpsimd queue: x0 tail-half, x1, then carries most output DMAs
            x1 = load_x(1, nc.gpsimd)
            xs = [x0, x1, x2, x3]

            # ---- PE HAM warm-up during the initial DMA wait ----
            nc.vector.memset(warm[:], 0)
            wps = pp.tile([128, 512], F32, tag="ps", name="ps")
            for i in range(8):
                nc.tensor.matmul(wps[:, :512], warm[:, :128], warm[:, 128:],
                                 start=True, stop=True)

            # strided candidate views of cls: [128, d, b(l/r-parity), s, 64]
            cls_r = clsT[:].rearrange("p (d s b x) -> p d b s x",
                                      d=DT, s=3, b=2, x=64)

            def patch_g1(ci, xa, h0, h1, g1s):
                c0, cw = CHUNKS[ci]
                for h in range(h0, h1):
                    ps = pp.tile([128, 512], F32, tag="ps", name="ps")
                    for d in range(DT):
                        nc.tensor.matmul(
                            ps[:, :cw],
                            w1T[:, (h * DT + d) * 128:(h * DT + d + 1) * 128],
                            xa[:, d * CW:d * CW + cw],
                            start=(d == 0), stop=(d == DT - 1))
                    g1 = g1p.tile([128, CW], BF16, tag="g1", name="g1")
                    nc.scalar.activation(g1[:, :cw], ps[:, :cw], AF.Gelu,
                                         bias=b1T[:, h:h + 1])
                    g1s.append(g1)
                return g1s

            def patch_g2(ci, g1s, eng_rr, dp_outer=False):
                c0, cw = CHUNKS[ci]

                def flush(dp, ps):
                    stg = op.tile([128, CW], F32, tag="ostg", name="ostg")
                    nc.vector.tensor_copy(stg[:, :cw], ps[:, :cw])
                    eng_rr[dp % len(eng_rr)].dma_start(
                        poutT_d[dp][:, c0:c0 + cw], stg[:, :cw])

                if dp_outer:
                    for dp in range(DT):
                        ps = pp.tile([128, 512], F32, tag="ps", name="ps")
                        for h in range(HT):
                            nc.tensor.matmul(
                                ps[:, :cw],
                                w2T[:, h * D + dp * 128:
                                    h * D + (dp + 1) * 128],
                                g1s[h][:, :cw],
                                start=(h == 0), stop=(h == HT - 1))
                        flush(dp, ps)
                else:
                    ps2 = [pp.tile([128, 512], F32, tag="ps", name="ps")
                           for _ in range(DT)]
                    for h in range(HT):
                        for dp in range(DT):
                            nc.tensor.matmul(
                                ps2[dp][:, :cw],
                                w2T[:, h * D + dp * 128:
                                    h * D + (dp + 1) * 128],
                                g1s[h][:, :cw],
                                start=(h == 0), stop=(h == HT - 1))
                    for dp in range(DT):
                        flush(dp, ps2[dp])

            # ---- chunk 0 (GEMM2 h-outer: w2 streams behind it) ----
            g1s0 = patch_g1(0, x0, 0, HT, [])
            patch_g2(0, g1s0, [nc.gpsimd])

            # ---- atom in-GEMM over candidate columns + gelu + mask ----
            gk = [gp.tile([128, 2 * NT], BF16, tag=f"g{k}", name=f"g{k}")
                  for k in range(KPA)]
            hk = [gp.tile([128, 2 * NT], BF16, tag=f"h{k}", name=f"h{k}")
                  for k in range(KPA)]
            for a in range(NA):
                for k in range(KPA):
                    ps = pp.tile([128, 512], F32, tag="ps", name="ps")
                    for d in range(DT):
                        if a < 3:
                            rhs = clsT[:, d * NT + 128 * a:
                                       d * NT + 128 * a + 128]
                        else:
                            rhs = cls_r[:, d, a - 3]
                        nc.tensor.matmul(
                            ps[:, :SA[a]],
                            ainT[d][:, a * HSH + k * 128:
                                    a * HSH + (k + 1) * 128],
                            rhs, start=(d == 0), stop=(d == DT - 1))
                    hl = a * KPA + k
                    nc.scalar.activation(gk[k][:, OFF[a]:OFF[a] + SA[a]],
                                         ps[:, :SA[a]], AF.Gelu,
                                         bias=ainbT[:, hl:hl + 1])
            for k in range(KPA):
                nc.vector.tensor_mul(hk[k][:], gk[k][:], mW[:])

            # ---- chunk 1 GEMM1 head (covers the atom gelu/mask latency) ----
            g1s1 = patch_g1(1, x1, 0, 6, [])

            # ---- atom out-GEMM ----
            # bank A: out-atoms 3,4 consume h~_{0,1,2} parity halves
            #   (compact cols: [b=0: s*64 | b=1: 192 + s*64])
            # bank B: out-atoms 0,1,2 consume h~_{3,4} slot-index columns
            #   (token-order cols 128*a)
            hA = [hk[k][:, :NT].rearrange("p (s b x) -> p b s x",
                                          s=3, b=2, x=64) for k in range(KPA)]
            hB = [hk[k][:, NT:].rearrange("p (u s x) -> p s u x",
                                          u=2, s=3, x=64) for k in range(KPA)]
            for dp in range(DT):
                psA = pp.tile([128, 512], F32, tag="ps", name="ps")
                psB = pp.tile([128, 512], F32, tag="ps", name="ps")
                for a, half in ((3, 0), (4, 1)):
                    for k in range(KPA):
                        nc.tensor.matmul(
                            psA[:, half * 192:(half + 1) * 192],
                            aoutT[a][:, k * D + dp * 128:
                                     k * D + (dp + 1) * 128],
                            hA[k][:, half],
                            start=(k == 0), stop=(k == KPA - 1))
                for a in range(3):
                    for k in range(KPA):
                        nc.tensor.matmul(
                            psB[:, a * 128:(a + 1) * 128],
                            aoutT[a][:, k * D + dp * 128:
                                     k * D + (dp + 1) * 128],
                            hB[k][:, a],
                            start=(k == 0), stop=(k == KPA - 1))
                stg = op.tile([128, CW], F32, tag="ostg", name="ostg")
                # token order (2s+b)*64+x <- psA[b*192+s*64+x] + psB[token]
                # (two DVE ops: the verifier rejects dual-PSUM-input ops)
                outv = stg[:, :NT].rearrange("p (s b x) -> p s b x",
                                             s=3, b=2, x=64)
                av = psA[:, :NT].rearrange("p (b s x) -> p s b x",
                                           b=2, s=3, x=64)
                nc.vector.tensor_copy(outv, av)
                nc.vector.tensor_add(stg[:, :NT], stg[:, :NT], psB[:, :NT])
                nc.gpsimd.dma_start(cpartT_d[dp], stg[:, :NT])

            # ---- remaining patch chunks ----
            patch_g1(1, x1, 6, HT, g1s1)
            patch_g2(1, g1s1, [nc.gpsimd])
            g1s2 = patch_g1(2, x2, 0, HT, [])
            patch_g2(2, g1s2, [nc.gpsimd])
            g1s3 = patch_g1(3, x3, 0, HT, [])
            patch_g2(3, g1s3, [nc.gpsimd, nc.sync, nc.scalar],
                     dp_outer=True)

    nc.compile()
    return nc


def _sigmoid(x):
    out = np.empty_like(x)
    pos = x >= 0
    out[pos] = 1.0 / (1.0 + np.exp(-x[pos]))
    ex = np.exp(x[~pos])
    out[~pos] = ex / (1.0 + ex)
    return out


def kernel(x, patch_w1, patch_b1, patch_w2, patch_b2, gate_delta,
           atom_in_w, atom_in_b, atom_out_w, atom_out_b):
    x = np.asarray(x, dtype=np.float32)
    patch_w1 = np.asarray(patch_w1, dtype=np.float32)
    patch_b1 = np.asarray(patch_b1, dtype=np.float32)
    patch_w2 = np.asarray(patch_w2, dtype=np.float32)
    patch_b2 = np.asarray(patch_b2, dtype=np.float32)
    gate_delta = np.asarray(gate_delta, dtype=np.float32)
    atom_in_w = np.asarray(atom_in_w, dtype=np.float32)
    atom_in_b = np.asarray(atom_in_b, dtype=np.float32)
    atom_out_w = np.asarray(atom_out_w, dtype=np.float32)
    atom_out_b = np.asarray(atom_out_b, dtype=np.float32)

    bf = ml_dtypes.bfloat16

    # ---- host routing (tiny), slot-major token order t' = n*64 + b ----
    cls3 = x[:, :NCLS, :]                                   # [B, 6, D]
    logits = np.einsum("bnd,nd->bn", cls3, gate_delta)      # [B, 6] f32
    choose_left = logits >= 0
    p_left = _sigmoid(logits)
    wgt = np.where(choose_left, p_left, 1.0 - p_left).astype(np.float32)
    keys = np.where(choose_left, LEFT_KEYS[None, :], RIGHT_KEYS[None, :])
    src_sm = (keys // NA).T.reshape(-1)                     # [384] slot-major
    dst_sm = (keys % NA).T.reshape(-1)
    w_sm = wgt.T.reshape(-1).astype(np.float32)

    # combined src-select * gate-weight mask over candidate columns
    SLOTS = {0: [0, 1], 1: [2, 3], 2: [4, 5], 3: [0, 2, 4], 4: [1, 3, 5]}
    m = np.zeros((2 * NT,), dtype=np.float32)
    for a in range(NA):
        cols = np.concatenate(
            [np.arange(s * 64, (s + 1) * 64) for s in SLOTS[a]])
        m[OFF[a]:OFF[a] + SA[a]] = np.where(
            src_sm[cols] == a, w_sm[cols], 0.0)
    mW_rep = np.ascontiguousarray(
        np.broadcast_to(m[None, :], (128, 2 * NT))).astype(np.float32)

    # ---- replicated tensors (partition-major packed) ----
    cls_sm = np.ascontiguousarray(cls3.transpose(1, 0, 2)).reshape(NT, D)
    # clsT[p, d*NT + t'] = cls_sm[t', d*128+p]
    clsT = np.ascontiguousarray(
        cls_sm.reshape(NT, DT, 128).transpose(2, 1, 0)
    ).reshape(128, DT * NT).astype(bf)
    # w1T[p, (h*6+d)*128 + m] = patch_w1[h*128+m, d*128+p]
    w1T = np.ascontiguousarray(
        patch_w1.reshape(HT, 128, DT, 128).transpose(3, 0, 2, 1)
    ).reshape(128, HT * DT * 128).astype(bf)
    b1T = np.ascontiguousarray(patch_b1.reshape(HT, 128).T)
    # w2T[p, h*D + dp*128 + m] = patch_w2[dp*128+m, h*128+p]
    w2T = np.ascontiguousarray(
        patch_w2.reshape(DT, 128, HT, 128).transpose(3, 2, 0, 1)
    ).reshape(128, HT * D).astype(bf)

    # ---- per-core tensors ----
    patch = x[:, NCLS:, :].reshape(NCORES, TPC, D)
    # xT[p, ci*DT*CW + d*CW + t] = patch[c][ci*CW+t, d*128+p]
    xT_all = np.ascontiguousarray(
        patch.reshape(NCORES, NCH, CW, DT, 128).transpose(0, 4, 1, 3, 2)
    ).reshape(NCORES, 128, NCH * DT * CW).astype(bf)

    ainT_all, ainbT_all, aoutT_all = [], [], []
    for c in range(NCORES):
        hsl = slice(HSH * c, HSH * (c + 1))
        # ainT[d, p, a*HSH + k*128 + m] = atom_in_w[a, hsl0 + k*128+m, d*128+p]
        ainT = np.ascontiguousarray(
            atom_in_w[:, hsl, :].reshape(NA, KPA, 128, DT, 128)
            .transpose(3, 4, 0, 1, 2)).reshape(DT, 128, NA * HSH).astype(bf)
        ainT_all.append(ainT)
        ainbT_all.append(np.ascontiguousarray(
            atom_in_b[:, hsl].reshape(HLT, 128).T))
        # aoutT[a, p, k*D + dp*128 + m] = atom_out_w[a, dp*128+m, hsl0+k*128+p]
        aoutT = np.ascontiguousarray(
            atom_out_w[:, :, hsl].reshape(NA, DT, 128, KPA, 128)
            .transpose(0, 4, 3, 1, 2)).reshape(NA, 128, KPA * D).astype(bf)
        aoutT_all.append(aoutT)

    in_maps = []
    for c in range(NCORES):
        in_maps.append({
            "xT": xT_all[c], "w1T": w1T, "b1T": b1T, "w2T": w2T,
            "clsT": clsT, "ainT": ainT_all[c], "ainbT": ainbT_all[c],
            "aoutT": aoutT_all[c], "mW": mW_rep,
        })

    nc = _CACHE.get("nc")
    if nc is None:
        nc = _build_program()
        _CACHE["nc"] = nc

    res = run_bass_kernel_spmd(nc, in_maps, core_ids=list(range(NCORES)))
    global LAST_RESULTS
    LAST_RESULTS = res

    # ---- host gather ----
    patch_out = np.empty((B, P, D), dtype=np.float32)
    for c in range(NCORES):
        poutT = res.results[c]["poutT"].reshape(D, TPC)
        patch_out[BPC * c:BPC * (c + 1)] = (
            poutT.T + patch_b2[None, :]).reshape(BPC, P, D)

    cpart = np.zeros((D, NT), dtype=np.float32)
    for c in range(NCORES):
        cpart += res.results[c]["cpartT"].reshape(D, NT)
    cls_out_sm = cpart.T + w_sm[:, None] * atom_out_b[dst_sm, :]
    cls_out = cls_out_sm.reshape(NCLS, B, D).transpose(1, 0, 2)

    return np.concatenate([cls_out, patch_out], axis=1)
